# revision 37
# baseline (speedup 1.0000x reference)
"""Expert-parallel MoE kernel for Trainium2 (8 NeuronCores).

Strategy (expert-parallel, per sharding hint):
  - Host: sort the T*top_k dispatch pairs by expert, scale each dispatched
    token by its gate score (gate folds into the linear map's input), pad
    each expert's token group to a fixed capacity CAP, lay out as [K, M]
    (pre-transposed for the PE's lhsT operand), cast to bf16.
  - Device (SPMD, core m owns experts 2m and 2m+1): Z_e = X_e^T.T @ W_e
    as tiled bf16 matmuls with fp32 PSUM accumulation.
  - Host: scatter Z rows back to dispatch pairs, sum top_k contributions,
    add the (gate-weighted) expert biases.

Schedule notes:
  - Every load->matmul edge pays ~2-2.5us of DMA completion latency (HBM
    write receipt before the semaphore fires), so loads are issued as
    fine (1-2 k-tile) DMAs in k order on the sync ring: the PE never
    waits on a coarse chunk's last byte, and each chunk's receipt hides
    under the next chunks' data.
  - The HAM clock gate holds the PE at 1.2 GHz until it sees ~3.4us of
    GAPLESS matmul activity. A chain of warmup matmuls on a zeroed tile
    bridges from body start to the first k0 semaphore, finished by tiny
    "gated" warmups that read the k0 x/w tiles themselves (so they wait
    on the same semaphores as the first real matmul) -- the real stream
    then runs fully warm at ~216ns per N=512 matmul.
  - The framework epilogue (event-semaphore teardown + notification
    flush, ~10us) is fixed and starts after the LAST body instruction;
    the final stores are therefore split into halves issued in parallel
    on both HWDGE rings right after their own PSUM copy, and the last
    k-tile runs m1-first so the tail chain is one copy + one issue deep.
"""

import numpy as np
import ml_dtypes

NUM_EXPERT = 16
D = 1024
TOP_K = 2
T = 2048
N_CORES = 8
EPC = NUM_EXPERT // N_CORES  # experts per core
CAP = 256                    # per-expert dispatch capacity (multiple of 128)
KT = D // 128                # contraction tiles
NT = D // 512                # output free-dim tiles (one PSUM bank each)
MT = CAP // 128              # token tiles
KC = 4                       # k-tiles per W DRAM chunk (tile granularity)
N_WARM = 18                  # PE warmup matmuls (bridge HAM ramp + DMA wait)

TRACE = False                # set by test harness to collect an NTFF profile
LAST_RESULT = None           # BassKernelResults of the most recent run

_NC = None


def _build_nc():
    from concourse import bacc, tile
    import concourse.mybir as mybir

    bf16 = mybir.dt.bfloat16
    f32 = mybir.dt.float32

    nc = bacc.Bacc("TRN2", target_bir_lowering=False, debug=False,
                   num_devices=N_CORES)
    # Flat chunk-major layouts: each DMA chunk is a [128, f] block whose
    # per-partition data is contiguous in DRAM (multi-KB descriptors).
    w = nc.declare_dram_parameter("w", [EPC, (KT // KC) * 128 * KC * D], bf16,
                                  isOutput=False)
    xt = nc.declare_dram_parameter("xt", [EPC, 128 * KT * CAP], bf16,
                                   isOutput=False)
    z = nc.declare_dram_parameter("z", [EPC, CAP, D], bf16, isOutput=True)

    with tile.TileContext(nc, num_cores=N_CORES) as tc:
        with (
            tc.tile_pool(name="wp", bufs=1) as wp,
            tc.tile_pool(name="xp", bufs=1) as xp,
            tc.tile_pool(name="pp", bufs=2, space="PSUM") as pp,
            tc.tile_pool(name="op", bufs=4) as op,
        ):
            # --- PE warmup: dummy matmuls on a zeroed tile keep the PE
            # busy through the HAM activity window while loads stream, so
            # the real matmuls run at 2.4 GHz instead of 1.2 GHz.
            warm = xp.tile([128, 512], bf16, name="warm", tag="warm")
            nc.vector.memset(warm[:], 0.0)
            wps = pp.tile([128, NT * 512], f32, name="ps0", tag="ps0")
            for i in range(N_WARM):
                nc.tensor.matmul(wps[:, :256], warm[:, :128],
                                 warm[:, :256], start=True, stop=True)

            # --- loads, in fine (1-2 k-tile) DMAs, k-ordered on the sync
            # ring (one x0 chunk rides the scalar ring, whose first issue
            # hides behind a ~1.5us ACT table load from scalar.copy)
            xts, wts = {}, {}
            xsrc, wsrc = {}, {}
            for e in range(EPC):
                xts[e] = xp.tile([128, KT * CAP], bf16,
                                 name=f"x{e}", tag=f"x{e}")
                xsrc[e] = xt[e].rearrange("(p f) -> p f", p=128)
                for c in range(KT // KC):
                    wtl = wp.tile([128, KC * D], bf16,
                                  name=f"w{e}_{c}", tag=f"w{e}_{c}")
                    wsrc[e, c] = w[e][c * (KC * 128 * D):
                                      (c + 1) * (KC * 128 * D)].rearrange(
                        "(p f) -> p f", p=128)
                    for kk in range(KC):
                        wts[e, c * KC + kk] = (wtl, kk)

            def wdma(eng, e, k0, kl):
                c, kk = k0 // KC, k0 % KC
                assert kk + kl <= KC
                tl = wts[e, k0][0]
                eng.dma_start(tl[:, kk * D:(kk + kl) * D],
                              wsrc[e, c][:, kk * D:(kk + kl) * D])

            def xdma(eng, e, k0, kl):
                eng.dma_start(xts[e][:, k0 * CAP:(k0 + kl) * CAP],
                              xsrc[e][:, k0 * CAP:(k0 + kl) * CAP])

            # One k-ordered stream on the sync ring: each chunk's ~2us
            # completion latency hides behind the previous chunks' data,
            # and late X chunks no longer dilute the early W stream.
            # Only x0's k2k3 rides the scalar ring (whose first issue sits
            # behind a ~1.5us ACT table load).
            wdma(nc.sync, 0, 0, 1)
            xdma(nc.sync, 0, 0, 2)
            wdma(nc.sync, 0, 1, 1)
            xdma(nc.scalar, 0, 2, 2)
            wdma(nc.sync, 0, 2, 1)
            wdma(nc.sync, 0, 3, 1)
            xdma(nc.sync, 0, 4, 4)
            wdma(nc.sync, 0, 4, 2)
            # from here the PE runs with <0.3us of margin over the data
            # stream, so k-singles only: a k-pair's first tile would wait
            # an extra 0.6us on its partner's data before the semaphore
            wdma(nc.sync, 0, 6, 1)
            wdma(nc.sync, 0, 7, 1)
            xdma(nc.sync, 1, 0, 2)
            wdma(nc.sync, 1, 0, 1)
            wdma(nc.sync, 1, 1, 1)
            xdma(nc.sync, 1, 2, 6)
            wdma(nc.sync, 1, 2, 1)
            wdma(nc.sync, 1, 3, 1)
            wdma(nc.sync, 1, 4, 2)
            wdma(nc.sync, 1, 6, 1)
            wdma(nc.sync, 1, 7, 1)

            # gated warmups: tiny matmuls that read the k0 x/w tiles, so
            # they wait on the same DMA semaphores as the first real
            # matmuls -- the PE stays busy through the load wait and the
            # HAM clock gate cannot re-throttle in the warmup->real gap
            for i in range(2):
                nc.tensor.matmul(wps[:, :128], xts[0][:, :128],
                                 warm[:, :128], start=True, stop=True)
            nc.tensor.matmul(wps[:, :128], warm[:, :128],
                             wts[0, 0][0][:, :128], start=True,
                             stop=True)

            # --- compute: k-outer so all (m, n) PSUM groups of an expert
            # accumulate in parallel and the PE consumes each k chunk as
            # it lands; [128, 1024] PSUM tiles span 2 banks (8 banks total
            # with bufs=2, so experts double-buffer).
            for e in range(EPC):
                pss = {}
                for m in range(MT):
                    pss[m] = pp.tile([128, NT * 512], f32,
                                     name=f"ps{m}", tag=f"ps{m}")
                last = e == EPC - 1
                for k in range(KT):
                    xtile = xts[e]
                    wtl, kk = wts[e, k]
                    # on the very last k-tile, do m1 first so its copies +
                    # store overlap m0's final matmuls
                    ms = reversed(range(MT)) if (last and k == KT - 1) \
                        else range(MT)
                    for m in ms:
                        for n in range(NT):
                            nc.tensor.matmul(
                                pss[m][:, n * 512:(n + 1) * 512],
                                xtile[:, k * CAP + m * 128:
                                      k * CAP + (m + 1) * 128],
                                wtl[:, kk * D + n * 512:
                                    kk * D + (n + 1) * 512],
                                start=(k == 0),
                                stop=(k == KT - 1),
                            )
                for m in (reversed(range(MT)) if last else range(MT)):
                    ot = op.tile([128, D], bf16)
                    # two halves on two engines -> each copy ~0.7us
                    nc.vector.tensor_copy(ot[:, :512], pss[m][:, :512])
                    nc.scalar.copy(ot[:, 512:], pss[m][:, 512:])
                    rows = z[e, m * 128:(m + 1) * 128, :]
                    if last:
                        # half-stores: each waits only on its own copy and
                        # the two issue in parallel on separate rings
                        nc.sync.dma_start(rows[:, :512], ot[:, :512])
                        nc.scalar.dma_start(rows[:, 512:], ot[:, 512:])
                    else:
                        # e0 stores on the sync ring: its FIFO keeps store
                        # data behind all remaining load data, so stores
                        # never steal SDMA time from the W stream
                        nc.sync.dma_start(rows, ot[:])
    nc.compile()
    return nc


def kernel(inp, gate_idx, gate_score, W, b):
    global _NC, LAST_RESULT
    from concourse.bass_utils import run_bass_kernel_spmd

    inp = np.ascontiguousarray(np.asarray(inp, dtype=np.float32))
    gi = np.asarray(gate_idx).astype(np.int64)
    gs = np.asarray(gate_score, dtype=np.float32)
    W = np.asarray(W, dtype=np.float32)
    b = np.asarray(b, dtype=np.float32)

    P = T * TOP_K
    fe = gi.reshape(P)
    fg = gs.reshape(P)
    tok = np.arange(P) // TOP_K

    order = np.argsort(fe, kind="stable")
    counts = np.bincount(fe, minlength=NUM_EXPERT)
    starts = np.zeros(NUM_EXPERT + 1, np.int64)
    np.cumsum(counts, out=starts[1:])
    rank = np.arange(P) - starts[fe[order]]
    ok = rank < CAP
    sel = order[ok]
    rnk = rank[ok]

    xpad = np.zeros((NUM_EXPERT, CAP, D), np.float32)
    xpad[fe[sel], rnk] = inp[tok[sel]] * fg[sel, None]
    # flat device layouts (must match _build_nc's chunking):
    # x: [128p, KT, CAP] per expert, per-partition contiguous
    xk = xpad.reshape(NUM_EXPERT, CAP, KT, 128)
    xt_dev = np.ascontiguousarray(
        xk.transpose(0, 3, 2, 1)).astype(ml_dtypes.bfloat16).reshape(
        NUM_EXPERT, -1)
    # w: per chunk c, [128p, KC, D], per-partition contiguous
    wk = W.reshape(NUM_EXPERT, KT, 128, D)
    w_parts = [
        np.ascontiguousarray(wk[:, c * KC:(c + 1) * KC].transpose(0, 2, 1, 3))
        .astype(ml_dtypes.bfloat16).reshape(NUM_EXPERT, -1)
        for c in range(KT // KC)
    ]
    w_dev = np.concatenate(w_parts, axis=1)

    if _NC is None:
        _NC = _build_nc()

    in_maps = [
        {"w": w_dev[c * EPC:(c + 1) * EPC],
         "xt": xt_dev[c * EPC:(c + 1) * EPC]}
        for c in range(N_CORES)
    ]
    res = run_bass_kernel_spmd(_NC, in_maps, list(range(N_CORES)),
                               trace=TRACE)
    LAST_RESULT = res
    zall = np.concatenate(
        [np.asarray(r["z"]).astype(np.float32) for r in res.results],
        axis=0)  # [E,CAP,D]

    zpairs = np.zeros((P, D), np.float32)
    zpairs[sel] = zall[fe[sel], rnk]
    # exact f32 fallback for over-capacity pairs (~2% of dispatches)
    overflow = order[~ok]
    if overflow.size:
        fe_o = fe[overflow]
        for e in np.unique(fe_o):
            pi = overflow[fe_o == e]
            zpairs[pi] = (inp[tok[pi]] * fg[pi, None]) @ W[e]

    y = zpairs.reshape(T, TOP_K, D).sum(axis=1)
    y += (gs[:, :, None] * b[gi]).sum(axis=1)
    return y.astype(np.float32)
